# revision 1
# baseline (speedup 1.0000x reference)
"""Trainium2 Bass kernel: NKQuantizer2 top-k masking (k=8).

reference:  kh = topk_hot(x, 8)          # [B,S,Q] 0/1 mask, top-8 per token
            out = einsum('bsq,eq->bse', kh, W)

Per token: out[t] = sum_{q in top8(x[t])} W[:, q] -- an 8-way embedding
gather-sum from W.T [Q, E].

Strategy (data-parallel over tokens across 8 cores, W.T bf16 in HBM):
  Per 128-token tile on each core:
    1. DMA x tile [128, 8192] f32 HBM->SBUF (4-deep prefetch)
    2. DVE Max8 -> top-8 values per token; DVE MaxIndex -> their indices
       (exact, ties -> first occurrence, matching jax.lax.top_k)
    3. 8 single-index indirect DMA gathers with CCE accumulate in the DMA
       datapath: acc[p, :] (+)= WT[idx8[p, j], :]  (bf16 in, f32 out)
    4. DMA acc -> out rows (f32)

Toolchain constraint: at most ONE semaphore wait per instruction. ALL DMAs
ride the single SWDGE FIFO queue (implicit cross-DMA ordering) in an
explicitly pinned pipeline order; buffer pools are sized so every
instruction has cross-proc dependencies on a single other proc.
"""

import numpy as np
import ml_dtypes

import concourse.bass as bass
import concourse.mybir as mybir
import concourse.tile as tile
from concourse.bass_utils import run_bass_kernel_spmd
from concourse.tile_rust import add_dep_helper

B, S, Q, E, TOPK = 4, 2048, 8192, 512, 8
N_CORES = 8
P = 128
T_TOTAL = B * S                 # 8192 tokens
T_CORE = T_TOTAL // N_CORES     # 1024 tokens per core

F32 = mybir.dt.float32
BF16 = mybir.dt.bfloat16
U32 = mybir.dt.uint32


def build_bass(t_core=T_CORE, q=Q, e=E):
    """Build the per-core Bass program (SPMD: same program on all cores)."""
    n_tiles = t_core // P
    xbufs = min(4, n_tiles)

    nc = bass.Bass(trn_type="TRN2", target_bir_lowering=False)
    x_d = nc.dram_tensor("x", [t_core, q], F32, kind="ExternalInput")
    wt_d = nc.dram_tensor("wt", [q, e], BF16, kind="ExternalInput")
    out_d = nc.dram_tensor("out", [t_core, e], F32, kind="ExternalOutput")

    fifo = []  # all SWDGE DMAs in intended FIFO order

    def swdge(dma):
        if fifo:
            add_dep_helper(dma.ins, fifo[-1].ins, False, "fifo order")
        fifo.append(dma)
        return dma

    with tile.TileContext(nc) as tc:
        with (
            tc.tile_pool(name="xpool", bufs=xbufs) as xpool,
            tc.tile_pool(name="spool", bufs=n_tiles) as spool,
            tc.tile_pool(name="ipool", bufs=n_tiles) as ipool,
            tc.tile_pool(name="gpool", bufs=n_tiles) as gpool,
        ):
            xts = [xpool.tile([P, q], F32, name="xt", tag="xt") for _ in range(xbufs)]
            idx8s, g8s, i_idxs, lastadds, ostores = [], [], [], [], []

            def emit_xload(i):
                xt = xts[i % xbufs]
                dma = nc.sync.dma_start(xt[:], x_d[i * P : (i + 1) * P, :])
                if i >= xbufs:
                    add_dep_helper(
                        dma.ins, i_idxs[i - xbufs].ins, True, "xt WAR"
                    )
                    # The WAW edge to the old x-load is redundant: the WAR on
                    # its readers already orders the writes at runtime.
                    dma.ins.try_remove_dependency(xls[i - xbufs].ins.name)
                return dma

            def emit_topk(i):
                xt = xts[i % xbufs]
                s8 = spool.tile([P, 8], F32, name="s8", tag="s8")
                nc.vector.max(out=s8[:], in_=xt[:])
                idx8 = ipool.tile([P, 8], U32, name="idx8", tag="idx8")
                i_idx = nc.vector.max_index(
                    out=idx8[:], in_max=s8[:], in_values=xt[:]
                )
                idx8s.append(idx8)
                i_idxs.append(i_idx)
                g8s.append(gpool.tile([P, e], F32, name="g8", tag="g8"))

            def emit_gather(i, j):
                swdge(
                    nc.gpsimd.indirect_dma_start(
                        out=g8s[i][:],
                        out_offset=None,
                        in_=wt_d[:],
                        in_offset=bass.IndirectOffsetOnAxis(
                            ap=idx8s[i][:, j : j + 1], axis=0
                        ),
                        compute_op=(
                            mybir.AluOpType.bypass
                            if j == 0
                            else mybir.AluOpType.add
                        ),
                    )
                )

            def emit_ostore(i):
                dma = swdge(
                    nc.gpsimd.dma_start(
                        out_d[i * P : (i + 1) * P, :], g8s[i][:]
                    )
                )
                ostores.append(dma)
                return dma

            # x-loads ride the HWDGE ring (8 loads, 8 fresh lanes, one wait
            # each); the SWDGE FIFO carries only gathers + stores, wave-major,
            # so tile i owns SWDGE lane i: its first gather carries the one
            # idx8 wait and every later gather/store's only wait is its
            # same-lane predecessor (= its accumulate-chain dependency).
            xls = []
            for i in range(n_tiles):
                xls.append(emit_xload(i))
                emit_topk(i)
            for j in range(TOPK):
                for i in range(n_tiles):
                    emit_gather(i, j)
            for i in range(n_tiles):
                emit_ostore(i)

            # Quiesce procs with single-wait SP nops so the kernel-tail
            # drains find their required ticks already observed.
            tail = xls + fifo[-10:] + i_idxs[-1:]
            for tgt in tail:
                n = nc.sync.nop()
                add_dep_helper(n.ins, tgt.ins, True, "tail quiesce")

    return nc


def _prep_wt(W: np.ndarray) -> np.ndarray:
    """W [e, q] f32 -> WT [q, e] bf16 contiguous."""
    return np.ascontiguousarray(W.T).astype(ml_dtypes.bfloat16)


_CACHED = {}


def _get_nc():
    if "nc" not in _CACHED:
        _CACHED["nc"] = build_bass()
    return _CACHED["nc"]


def kernel(x: np.ndarray, W: np.ndarray) -> np.ndarray:
    x = np.asarray(x, dtype=np.float32)
    W = np.asarray(W, dtype=np.float32)
    assert x.shape == (B, S, Q) and W.shape == (E, Q)

    nc = _get_nc()
    xf = x.reshape(T_TOTAL, Q)
    WT = _prep_wt(W)
    in_maps = [
        {
            "x": np.ascontiguousarray(xf[c * T_CORE : (c + 1) * T_CORE]),
            "wt": WT,
        }
        for c in range(N_CORES)
    ]
    res = run_bass_kernel_spmd(nc, in_maps, core_ids=list(range(N_CORES)))
    out = np.concatenate([r["out"] for r in res.results], axis=0)
    return np.ascontiguousarray(out.reshape(B, S, E).astype(np.float32))

